# revision 3
# baseline (speedup 1.0000x reference)
"""DYSPN attention-conv kernel for Trainium2 (8 NeuronCores, batch-parallel).

Math (the reference's unfold/fold pair collapses algebraically):
  per image, per tap k=(i,j) != center, ring r = INDEX[i,j], dy = 3-i, dx = 3-j:
    z_k[y,x]  = att_r[y,x] * aff_k[y,x]
    U[y,x]    = sum_k z_k[y,x]                       (S_ppt - att3)
    A[y,x]    = sum_k |z_k[y,x]|                     (S_prime - att3; att >= 0)
    T[y,x]    = sum_k z_k[y+dy, x+dx]  (in-image)    (fold7(z))
  out = r * ((T+att3)*cs - (U+att3)*co) + co,  r = 1/(A+att3+eps)

Layout: batch 16 -> 2 images/core. Row-pair layout: partition p holds image
rows {2p, 2p+1}; each affinity tap plane is host-padded to [128, 2, 262]
(3 zero cols each side per half-row) so column shifts read DRAM-zeroed
guards and every DMA descriptor is one contiguous 2096B run.
  - DVE: z = att*aff (ring-broadcast tensor_tensor, in-place, fp32r out)
  - ACT: |z| -> bf16 az tiles
  - PE : U/A/T as banded-"identity" matmuls accumulating in PSUM
         (one full bank [128,2,256] per accumulator; even row shifts via
         band diagonal on pairs, odd shifts via two per-half matmuls)
  - DVE: epilogue; out stores + cs/co loads on the scalar HWDGE ring so
         the sync ring streams affinity without head-of-line blocking.
"""
import sys

sys.path.insert(0, "/opt/trn_rl_repo")

import numpy as np

import concourse.bass as bass  # noqa: F401  (registers engines)
import concourse.tile as tile
from concourse import bacc, mybir
from concourse.bass_utils import run_bass_kernel_spmd

FP32 = mybir.dt.float32
FP32R = mybir.dt.float32r
BF16 = mybir.dt.bfloat16

N_CORES = 8
B_FULL = 16
B_CORE = B_FULL // N_CORES  # 2 images per core
H = W = 256
K = 7
NTAP = 48                 # 49 minus center
PAD = 3                   # zero guard cols per side of each half-row
HWP = 2 * PAD + W         # 262: padded half-row width
BANDW = 132               # band[p, q] = 1 iff q == p + C0
C0 = 2
EPS = 1e-6

CHUNK_SIZES = [2, 2, 4, 8, 8, 8, 8, 8]
CHUNKS = []
_lo = 0
for _cs in CHUNK_SIZES:
    CHUNKS.append((_lo, _lo + _cs))
    _lo += _cs

# ring index of each tap in the 7x7 window (center marked 3, excluded)
_INDEX = np.array([0, 0, 0, 0, 0, 0, 0,
                   0, 1, 1, 1, 1, 1, 0,
                   0, 1, 2, 2, 2, 1, 0,
                   0, 1, 2, 3, 2, 1, 0,
                   0, 1, 2, 2, 2, 1, 0,
                   0, 1, 1, 1, 1, 1, 0,
                   0, 0, 0, 0, 0, 0, 0], dtype=np.int64).reshape(7, 7)

TAPS = []  # (t, ring, dy, dx) in DRAM plane order, center skipped
for i in range(K):
    for j in range(K):
        if i == 3 and j == 3:
            continue
        TAPS.append((len(TAPS), int(_INDEX[i, j]), 3 - i, 3 - j))

# maximal runs of taps (in t-order) sharing one ring -> one DVE mul each
RUNS = []  # [t_lo, t_hi, ring]
for t, r, dy, dx in TAPS:
    if RUNS and RUNS[-1][2] == r and RUNS[-1][1] == t:
        RUNS[-1][1] = t + 1
    else:
        RUNS.append([t, t + 1, r])
RUNS = [tuple(x) for x in RUNS]


def _band_matrix() -> np.ndarray:
    band = np.zeros((128, BANDW), dtype=np.float32)
    for p in range(128):
        band[p, p + C0] = 1.0
    return band


def _build():
    nc = bacc.Bacc("TRN2", target_bir_lowering=False, debug=False,
                   num_devices=N_CORES)
    # float32r: same 4-byte layout as fp32 (numpy sees float32). Static BIR
    # typing only — the in-place z multiply re-produces every element rounded
    # to fp32r before any matmul consumes it.
    aff = nc.dram_tensor("affinity", [B_CORE, NTAP, 128, 2, HWP], FP32R,
                         kind="ExternalInput").ap()
    att = nc.dram_tensor("attention", [B_CORE, 4, 128, 2, W], FP32,
                         kind="ExternalInput").ap()
    cs = nc.dram_tensor("current_segmentation", [B_CORE, 128, 2, W], FP32,
                        kind="ExternalInput").ap()
    co = nc.dram_tensor("coarse_segmentation", [B_CORE, 128, 2, W], FP32,
                        kind="ExternalInput").ap()
    band = nc.dram_tensor("band", [128, BANDW], FP32, kind="ExternalInput").ap()
    out = nc.dram_tensor("out", [B_CORE, 128, 2 * W], FP32,
                         kind="ExternalOutput").ap()

    with tile.TileContext(nc) as tc:
        with tc.tile_pool(name="const", bufs=1) as cpool, \
             tc.tile_pool(name="z2", bufs=4) as zp2, \
             tc.tile_pool(name="z4", bufs=2) as zp4, \
             tc.tile_pool(name="z8", bufs=6) as zp8, \
             tc.tile_pool(name="azp", bufs=3) as azpool, \
             tc.tile_pool(name="inp", bufs=2) as ipool, \
             tc.tile_pool(name="ep", bufs=2) as epool, \
             tc.tile_pool(name="ps", bufs=2, space="PSUM") as pspool:

            bandf = cpool.tile([128, BANDW], FP32)
            nc.scalar.dma_start(out=bandf[:], in_=band[:, :])
            bandr = cpool.tile([128, BANDW], FP32R)
            nc.vector.tensor_copy(bandr[:], bandf[:])
            identb = cpool.tile([128, 128], BF16)
            nc.vector.tensor_copy(identb[:], bandf[:, C0:C0 + 128])
            identr = bandr[:, C0:C0 + 128]

            zpools = {2: zp2, 4: zp4, 8: zp8}

            for img in range(B_CORE):
                attf = ipool.tile([128, 4, 2, W], FP32, tag="attf")
                nc.sync.dma_start(out=attf[:, 0], in_=att[img, 0])
                nc.sync.dma_start(out=attf[:, 3], in_=att[img, 3])
                cst = ipool.tile([128, 2, W], FP32, tag="cst")
                nc.scalar.dma_start(out=cst[:], in_=cs[img])
                cot = ipool.tile([128, 2, W], FP32, tag="cot")
                nc.scalar.dma_start(out=cot[:], in_=co[img])
                att3r = ipool.tile([128, 2, W], FP32R, tag="att3r")
                nc.scalar.activation(att3r[:], attf[:, 3],
                                     mybir.ActivationFunctionType.Copy)

                # affinity chunk DMAs: graded sizes so compute starts early;
                # all on the sync HWDGE ring, nothing else ever queues there
                zts = []
                for ci, (lo, hi) in enumerate(CHUNKS):
                    cn = hi - lo
                    zt = zpools[cn].tile([128, cn, 2, HWP], FP32R, tag="zt")
                    zts.append(zt)
                    nc.sync.dma_start(
                        out=zt[:],
                        in_=aff[img, lo:hi].transpose([1, 0, 2, 3]))
                    if ci == 0:
                        nc.sync.dma_start(out=attf[:, 1], in_=att[img, 1])
                        nc.sync.dma_start(out=attf[:, 2], in_=att[img, 2])

                # PSUM accumulators, opened with the +att3 tap
                psU = pspool.tile([128, 2, W], FP32, tag="U")
                psA = pspool.tile([128, 2, W], FP32, tag="A")
                psT = pspool.tile([128, 2, W], FP32, tag="T")
                nc.tensor.matmul(out=psU[:], lhsT=identr, rhs=att3r[:],
                                 start=True, stop=False)
                nc.tensor.matmul(out=psA[:], lhsT=identr, rhs=att3r[:],
                                 start=True, stop=False)
                nc.tensor.matmul(out=psT[:], lhsT=identr, rhs=att3r[:],
                                 start=True, stop=False)

                for ci, (lo, hi) in enumerate(CHUNKS):
                    zt = zts[ci]
                    last = hi == NTAP
                    # z = att_r * aff (DVE, in-place, fp32r out)
                    for rlo, rhi, r in RUNS:
                        a, b = max(rlo, lo), min(rhi, hi)
                        if a >= b:
                            continue
                        zs = zt[:, a - lo:b - lo, :, PAD:PAD + W]
                        nc.vector.tensor_tensor(
                            out=zs,
                            in0=zs.bitcast(FP32),
                            in1=attf[:, r].unsqueeze(1).broadcast_to(
                                [128, b - a, 2, W]),
                            op=mybir.AluOpType.mult)
                    # |z| -> bf16 (ACT)
                    azt = azpool.tile([128, 8, 2, W], BF16, tag="az")
                    nc.scalar.activation(azt[:, 0:hi - lo],
                                         zt[:, :, :, PAD:PAD + W],
                                         mybir.ActivationFunctionType.Abs)
                    # U (and in the last chunk also A) close before T so the
                    # epilogue's U/A-dependent ops overlap the tail T matmuls
                    for t, r, dy, dx in TAPS[lo:hi]:
                        nc.tensor.matmul(out=psU[:], lhsT=identr,
                                         rhs=zt[:, t - lo, :, PAD:PAD + W],
                                         start=False, stop=(t == NTAP - 1))
                    if last:
                        for t, r, dy, dx in TAPS[lo:hi]:
                            nc.tensor.matmul(out=psA[:], lhsT=identb,
                                             rhs=azt[:, t - lo],
                                             start=False, stop=(t == NTAP - 1))
                    for t, r, dy, dx in TAPS[lo:hi]:
                        if dy % 2 == 0:
                            s = dy // 2
                            nc.tensor.matmul(
                                out=psT[:],
                                lhsT=bandr[:, C0 + s:C0 + s + 128],
                                rhs=zt[:, t - lo, :, PAD + dx:PAD + dx + W],
                                start=False, stop=False)
                        else:
                            for h in (0, 1):
                                s = (dy - 1) // 2 if h == 0 else (dy + 1) // 2
                                nc.tensor.matmul(
                                    out=psT[:, h],
                                    lhsT=bandr[:, C0 + s:C0 + s + 128],
                                    rhs=zt[:, t - lo, 1 - h,
                                           PAD + dx:PAD + dx + W],
                                    start=False,
                                    stop=(t == NTAP - 1 and h == 1))
                    if not last:
                        for t, r, dy, dx in TAPS[lo:hi]:
                            nc.tensor.matmul(out=psA[:], lhsT=identb,
                                             rhs=azt[:, t - lo],
                                             start=False, stop=False)

                # ---- epilogue (DVE; r = 1/(A+att3+eps)) ----
                e = epool.tile([128, 2, W], FP32, tag="e")
                nc.vector.tensor_scalar_add(e[:], psA[:], EPS)
                nc.vector.reciprocal(e[:], e[:])
                m2 = epool.tile([128, 2, W], FP32, tag="m2")
                nc.vector.tensor_mul(m2[:], psU[:], cot[:])
                nc.vector.tensor_mul(m2[:], m2[:], e[:])
                nc.vector.tensor_sub(m2[:], m2[:], cot[:])
                nc.vector.tensor_mul(e[:], e[:], cst[:])
                m1 = epool.tile([128, 2, W], FP32, tag="m1")
                nc.vector.tensor_mul(m1[:], psT[:], e[:])
                nc.vector.tensor_sub(m1[:], m1[:], m2[:])
                nc.scalar.dma_start(out=out[img], in_=m1[:])

    nc.compile()
    return nc


_NC_CACHE = None


def _get_nc():
    global _NC_CACHE
    if _NC_CACHE is None:
        _NC_CACHE = _build()
    return _NC_CACHE


_KEEP = [k for k in range(49) if k != 24]


def run(inputs: dict, trace: bool = False):
    """Run on 8 NeuronCores; returns (out [16,1,256,256], BassKernelResults)."""
    aff = np.asarray(inputs["affinity"], dtype=np.float32)
    att = np.asarray(inputs["attention"], dtype=np.float32)
    cs = np.asarray(inputs["current_segmentation"], dtype=np.float32)
    co = np.asarray(inputs["coarse_segmentation"], dtype=np.float32)
    band = _band_matrix()

    nc = _get_nc()
    in_maps = []
    for c in range(N_CORES):
        s = slice(c * B_CORE, (c + 1) * B_CORE)
        affp = np.zeros((B_CORE, NTAP, 128, 2, HWP), np.float32)
        affp[..., PAD:PAD + W] = aff[s][:, _KEEP].reshape(
            B_CORE, NTAP, 128, 2, W)
        in_maps.append({
            "affinity": affp,
            "attention": np.ascontiguousarray(att[s]).reshape(
                B_CORE, 4, 128, 2, W),
            "current_segmentation": np.ascontiguousarray(cs[s]).reshape(
                B_CORE, 128, 2, W),
            "coarse_segmentation": np.ascontiguousarray(co[s]).reshape(
                B_CORE, 128, 2, W),
            "band": band,
        })
    last_err = None
    for attempt in range(3):
        try:
            res = run_bass_kernel_spmd(nc, in_maps, list(range(N_CORES)),
                                       trace=trace)
            break
        except Exception as e:  # transient NRT_EXEC_UNIT_UNRECOVERABLE flakes
            last_err = e
            import time
            time.sleep(10)
    else:
        raise last_err
    full = np.concatenate(
        [res.results[c]["out"].reshape(B_CORE, 1, H, W) for c in range(N_CORES)],
        axis=0)
    return full, res


def kernel(**inputs) -> np.ndarray:
    out, _ = run(inputs, trace=False)
    return out


# revision 4
# speedup vs baseline: 1.0184x; 1.0184x over previous
"""DYSPN attention-conv kernel for Trainium2 (8 NeuronCores, batch-parallel).

Math (the reference's unfold/fold pair collapses algebraically):
  per image, per tap k=(i,j) != center, ring r = INDEX[i,j], dy = 3-i, dx = 3-j:
    z_k[y,x]  = att_r[y,x] * aff_k[y,x]
    U[y,x]    = sum_k z_k[y,x]                       (S_ppt - att3)
    A[y,x]    = sum_k |z_k[y,x]|                     (S_prime - att3; att >= 0)
    T[y,x]    = sum_k z_k[y+dy, x+dx]  (in-image)    (fold7(z))
  out = r * ((T+att3)*cs - (U+att3)*co) + co,  r = 1/(A+att3+eps)

Layout: batch 16 -> 2 images/core. Row-pair layout: partition p holds image
rows {2p, 2p+1}; each affinity tap plane is host-padded to [128, 2, 262]
(3 zero cols each side per half-row) so column shifts read DRAM-zeroed
guards and every DMA descriptor is one contiguous 2096B run.
  - DVE: z = att*aff (ring-broadcast tensor_tensor, in-place, fp32r out)
  - ACT: |z| -> bf16 az tiles; PSUM A + eps copy
  - PE : U/A/T as banded-"identity" matmuls accumulating in PSUM
         (one full bank [128,2,256] per accumulator; even row shifts via
         band diagonal on pairs, odd shifts via two per-half matmuls)
  - DVE: epilogue, hoisted off the z-mult stream (img0's epilogue is issued
         mid-img1 so it never head-of-line-blocks img1's multiplies);
         out stores + cs/co loads ride the scalar HWDGE ring so the sync
         ring streams affinity uninterrupted.
"""
import sys

sys.path.insert(0, "/opt/trn_rl_repo")

import numpy as np

import concourse.bass as bass  # noqa: F401  (registers engines)
import concourse.tile as tile
from concourse import bacc, mybir
from concourse.bass_utils import run_bass_kernel_spmd

FP32 = mybir.dt.float32
FP32R = mybir.dt.float32r
BF16 = mybir.dt.bfloat16

N_CORES = 8
B_FULL = 16
B_CORE = B_FULL // N_CORES  # 2 images per core
H = W = 256
K = 7
NTAP = 48                 # 49 minus center
PAD = 3                   # zero guard cols per side of each half-row
HWP = 2 * PAD + W         # 262: padded half-row width
BANDW = 132               # band[p, q] = 1 iff q == p + C0
C0 = 2
EPS = 1e-6

CHUNK_SIZES = [2, 2] + [4] * 11
CHUNKS = []
_lo = 0
for _cs in CHUNK_SIZES:
    CHUNKS.append((_lo, _lo + _cs))
    _lo += _cs
EPI_AT = 3  # issue previous image's epilogue after this chunk index of the next

# ring index of each tap in the 7x7 window (center marked 3, excluded)
_INDEX = np.array([0, 0, 0, 0, 0, 0, 0,
                   0, 1, 1, 1, 1, 1, 0,
                   0, 1, 2, 2, 2, 1, 0,
                   0, 1, 2, 3, 2, 1, 0,
                   0, 1, 2, 2, 2, 1, 0,
                   0, 1, 1, 1, 1, 1, 0,
                   0, 0, 0, 0, 0, 0, 0], dtype=np.int64).reshape(7, 7)

TAPS = []  # (t, ring, dy, dx) in DRAM plane order, center skipped
for i in range(K):
    for j in range(K):
        if i == 3 and j == 3:
            continue
        TAPS.append((len(TAPS), int(_INDEX[i, j]), 3 - i, 3 - j))

# maximal runs of taps (in t-order) sharing one ring -> one DVE mul each
RUNS = []  # [t_lo, t_hi, ring]
for t, r, dy, dx in TAPS:
    if RUNS and RUNS[-1][2] == r and RUNS[-1][1] == t:
        RUNS[-1][1] = t + 1
    else:
        RUNS.append([t, t + 1, r])
RUNS = [tuple(x) for x in RUNS]


def _band_matrix() -> np.ndarray:
    band = np.zeros((128, BANDW), dtype=np.float32)
    for p in range(128):
        band[p, p + C0] = 1.0
    return band


def _build():
    nc = bacc.Bacc("TRN2", target_bir_lowering=False, debug=False,
                   num_devices=N_CORES)
    # float32r: same 4-byte layout as fp32 (numpy sees float32). Static BIR
    # typing only — the in-place z multiply re-produces every element rounded
    # to fp32r before any matmul consumes it.
    aff = nc.dram_tensor("affinity", [B_CORE, NTAP, 128, 2, HWP], FP32R,
                         kind="ExternalInput").ap()
    att = nc.dram_tensor("attention", [B_CORE, 4, 128, 2, W], FP32,
                         kind="ExternalInput").ap()
    cs = nc.dram_tensor("current_segmentation", [B_CORE, 128, 2, W], FP32,
                        kind="ExternalInput").ap()
    co = nc.dram_tensor("coarse_segmentation", [B_CORE, 128, 2, W], FP32,
                        kind="ExternalInput").ap()
    band = nc.dram_tensor("band", [128, BANDW], FP32, kind="ExternalInput").ap()
    out = nc.dram_tensor("out", [B_CORE, 128, 2, W], FP32,
                         kind="ExternalOutput").ap()

    with tile.TileContext(nc) as tc:
        with tc.tile_pool(name="const", bufs=1) as cpool, \
             tc.tile_pool(name="z2", bufs=4) as zp2, \
             tc.tile_pool(name="z4", bufs=10) as zp4, \
             tc.tile_pool(name="azp", bufs=4) as azpool, \
             tc.tile_pool(name="inp", bufs=2) as ipool, \
             tc.tile_pool(name="ep", bufs=2) as epool, \
             tc.tile_pool(name="ps", bufs=2, space="PSUM") as pspool:

            bandf = cpool.tile([128, BANDW], FP32)
            nc.scalar.dma_start(out=bandf[:], in_=band[:, :])
            bandr = cpool.tile([128, BANDW], FP32R)
            nc.vector.tensor_copy(bandr[:], bandf[:])
            identb = cpool.tile([128, 128], BF16)
            nc.vector.tensor_copy(identb[:], bandf[:, C0:C0 + 128])
            identr = bandr[:, C0:C0 + 128]

            zpools = {2: zp2, 4: zp4}
            st = {}  # per-image state for the deferred epilogue

            def epilogue(img):
                s = st[img]
                e = epool.tile([128, 2, W], FP32, tag="e")
                nc.scalar.activation(e[:], s["A"][:],
                                     mybir.ActivationFunctionType.Copy,
                                     bias=EPS)
                nc.vector.reciprocal(e[:], e[:])
                m2 = epool.tile([128, 2, W], FP32, tag="m2")
                nc.vector.tensor_mul(m2[:], s["U"][:], s["cot"][:])
                nc.vector.tensor_mul(m2[:], m2[:], e[:])
                nc.vector.tensor_sub(m2[:], m2[:], s["cot"][:])
                nc.vector.tensor_mul(e[:], e[:], s["cst"][:])
                m1 = epool.tile([128, 2, W], FP32, tag="m1")
                nc.vector.tensor_mul(m1[:], s["T"][:], e[:])
                nc.vector.tensor_sub(m1[:], m1[:], m2[:])
                nc.scalar.dma_start(out=out[img], in_=m1[:])

            for img in range(B_CORE):
                attf = ipool.tile([128, 4, 2, W], FP32, tag="attf")
                nc.sync.dma_start(out=attf[:, 0], in_=att[img, 0])
                nc.sync.dma_start(out=attf[:, 3], in_=att[img, 3])
                cst = ipool.tile([128, 2, W], FP32, tag="cst")
                nc.scalar.dma_start(out=cst[:], in_=cs[img])
                cot = ipool.tile([128, 2, W], FP32, tag="cot")
                nc.scalar.dma_start(out=cot[:], in_=co[img])
                att3r = ipool.tile([128, 2, W], FP32R, tag="att3r")
                nc.scalar.activation(att3r[:], attf[:, 3],
                                     mybir.ActivationFunctionType.Copy)

                # affinity chunk DMAs: graded sizes so compute starts early;
                # all on the sync HWDGE ring, nothing else ever queues there
                zts = []
                for ci, (lo, hi) in enumerate(CHUNKS):
                    cn = hi - lo
                    zt = zpools[cn].tile([128, cn, 2, HWP], FP32R, tag="zt")
                    zts.append(zt)
                    nc.sync.dma_start(
                        out=zt[:],
                        in_=aff[img, lo:hi].transpose([1, 0, 2, 3]))
                    if ci == 0:
                        nc.sync.dma_start(out=attf[:, 1], in_=att[img, 1])
                        nc.sync.dma_start(out=attf[:, 2], in_=att[img, 2])

                # PSUM accumulators, opened with the +att3 tap
                psU = pspool.tile([128, 2, W], FP32, tag="U")
                psA = pspool.tile([128, 2, W], FP32, tag="A")
                psT = pspool.tile([128, 2, W], FP32, tag="T")
                st[img] = {"U": psU, "A": psA, "T": psT, "cst": cst,
                           "cot": cot}
                nc.tensor.matmul(out=psU[:], lhsT=identr, rhs=att3r[:],
                                 start=True, stop=False)
                nc.tensor.matmul(out=psA[:], lhsT=identr, rhs=att3r[:],
                                 start=True, stop=False)
                nc.tensor.matmul(out=psT[:], lhsT=identr, rhs=att3r[:],
                                 start=True, stop=False)

                for ci, (lo, hi) in enumerate(CHUNKS):
                    zt = zts[ci]
                    # z = att_r * aff (DVE, in-place, fp32r out)
                    for rlo, rhi, r in RUNS:
                        a, b = max(rlo, lo), min(rhi, hi)
                        if a >= b:
                            continue
                        zs = zt[:, a - lo:b - lo, :, PAD:PAD + W]
                        nc.vector.tensor_tensor(
                            out=zs,
                            in0=zs.bitcast(FP32),
                            in1=attf[:, r].unsqueeze(1).broadcast_to(
                                [128, b - a, 2, W]),
                            op=mybir.AluOpType.mult)
                    # |z| -> bf16 (ACT)
                    azt = azpool.tile([128, 4, 2, W], BF16, tag="az")
                    nc.scalar.activation(azt[:, 0:hi - lo],
                                         zt[:, :, :, PAD:PAD + W],
                                         mybir.ActivationFunctionType.Abs)
                    for t, r, dy, dx in TAPS[lo:hi]:
                        nc.tensor.matmul(out=psU[:], lhsT=identr,
                                         rhs=zt[:, t - lo, :, PAD:PAD + W],
                                         start=False, stop=(t == NTAP - 1))
                    for t, r, dy, dx in TAPS[lo:hi]:
                        if dy % 2 == 0:
                            s = dy // 2
                            nc.tensor.matmul(
                                out=psT[:],
                                lhsT=bandr[:, C0 + s:C0 + s + 128],
                                rhs=zt[:, t - lo, :, PAD + dx:PAD + dx + W],
                                start=False, stop=False)
                        else:
                            for h in (0, 1):
                                s = (dy - 1) // 2 if h == 0 else (dy + 1) // 2
                                nc.tensor.matmul(
                                    out=psT[:, h],
                                    lhsT=bandr[:, C0 + s:C0 + s + 128],
                                    rhs=zt[:, t - lo, 1 - h,
                                           PAD + dx:PAD + dx + W],
                                    start=False,
                                    stop=(t == NTAP - 1 and h == 1))
                    for t, r, dy, dx in TAPS[lo:hi]:
                        nc.tensor.matmul(out=psA[:], lhsT=identb,
                                         rhs=azt[:, t - lo],
                                         start=False, stop=(t == NTAP - 1))
                    if ci == EPI_AT and img > 0:
                        epilogue(img - 1)

            epilogue(B_CORE - 1)

    nc.compile()
    return nc


_NC_CACHE = None


def _get_nc():
    global _NC_CACHE
    if _NC_CACHE is None:
        _NC_CACHE = _build()
    return _NC_CACHE


_KEEP = [k for k in range(49) if k != 24]


def run(inputs: dict, trace: bool = False):
    """Run on 8 NeuronCores; returns (out [16,1,256,256], BassKernelResults)."""
    aff = np.asarray(inputs["affinity"], dtype=np.float32)
    att = np.asarray(inputs["attention"], dtype=np.float32)
    cs = np.asarray(inputs["current_segmentation"], dtype=np.float32)
    co = np.asarray(inputs["coarse_segmentation"], dtype=np.float32)
    band = _band_matrix()

    nc = _get_nc()
    in_maps = []
    for c in range(N_CORES):
        s = slice(c * B_CORE, (c + 1) * B_CORE)
        affp = np.zeros((B_CORE, NTAP, 128, 2, HWP), np.float32)
        affp[..., PAD:PAD + W] = aff[s][:, _KEEP].reshape(
            B_CORE, NTAP, 128, 2, W)
        in_maps.append({
            "affinity": affp,
            "attention": np.ascontiguousarray(att[s]).reshape(
                B_CORE, 4, 128, 2, W),
            "current_segmentation": np.ascontiguousarray(cs[s]).reshape(
                B_CORE, 128, 2, W),
            "coarse_segmentation": np.ascontiguousarray(co[s]).reshape(
                B_CORE, 128, 2, W),
            "band": band,
        })
    last_err = None
    for attempt in range(3):
        try:
            res = run_bass_kernel_spmd(nc, in_maps, list(range(N_CORES)),
                                       trace=trace)
            break
        except Exception as e:  # transient NRT_EXEC_UNIT_UNRECOVERABLE flakes
            last_err = e
            import time
            time.sleep(10)
    else:
        raise last_err
    full = np.concatenate(
        [res.results[c]["out"].reshape(B_CORE, 1, H, W) for c in range(N_CORES)],
        axis=0)
    return full, res


def kernel(**inputs) -> np.ndarray:
    out, _ = run(inputs, trace=False)
    return out


# revision 6
# speedup vs baseline: 1.0689x; 1.0496x over previous
"""DYSPN attention-conv kernel for Trainium2 (8 NeuronCores, batch-parallel).

Math (the reference's unfold/fold pair collapses algebraically):
  per image, per tap k=(i,j) != center, ring r = INDEX[i,j], dy = 3-i, dx = 3-j:
    z_k[y,x]  = att_r[y,x] * aff_k[y,x]
    U[y,x]    = sum_k z_k[y,x]                       (S_ppt - att3)
    A[y,x]    = sum_k |z_k[y,x]|                     (S_prime - att3; att >= 0)
    T[y,x]    = sum_k z_k[y+dy, x+dx]  (in-image)    (fold7(z))
  out = r * ((T+att3)*cs - (U+att3)*co) + co,  r = 1/(A+att3+eps)

Layout: batch 16 -> 2 images/core. Row-pair layout: partition p holds image
rows {2p, 2p+1}; each affinity tap plane is host-padded to [128, 2, 262]
(3 zero cols each side per half-row) so column shifts read DRAM-zeroed
guards and every DMA descriptor is one contiguous 2096B run.
  - DVE: z = att*aff (ring-broadcast tensor_tensor, in-place, fp32r out)
  - ACT: |z| -> bf16 az tiles; PSUM A + eps copy
  - PE : U/A/T as banded-"identity" matmuls accumulating in PSUM
         (one full bank [128,2,256] per accumulator; even row shifts via
         band diagonal on pairs, odd shifts via two per-half matmuls)
  - DVE: epilogue, hoisted off the z-mult stream (img0's epilogue is issued
         mid-img1 so it never head-of-line-blocks img1's multiplies);
         out stores + cs/co loads ride the scalar HWDGE ring so the sync
         ring streams affinity uninterrupted.
"""
import sys

sys.path.insert(0, "/opt/trn_rl_repo")

import numpy as np

import concourse.bass as bass  # noqa: F401  (registers engines)
import concourse.tile as tile
from concourse import bacc, mybir
from concourse.bass_utils import run_bass_kernel_spmd

FP32 = mybir.dt.float32
FP32R = mybir.dt.float32r
BF16 = mybir.dt.bfloat16

N_CORES = 8
B_FULL = 16
B_CORE = B_FULL // N_CORES  # 2 images per core
H = W = 256
K = 7
NTAP = 48                 # 49 minus center
PAD = 3                   # zero guard cols per side of each half-row
HWP = 2 * PAD + W         # 262: padded half-row width
BANDW = 132               # band[p, q] = 1 iff q == p + C0
C0 = 2
EPS = 1e-6

CHUNK_SIZES = [2, 2] + [4] * 10 + [2, 2]
CHUNKS = []
_lo = 0
for _cs in CHUNK_SIZES:
    CHUNKS.append((_lo, _lo + _cs))
    _lo += _cs
EPI_AT = 3  # issue previous image's epilogue after this chunk index of the next

# ring index of each tap in the 7x7 window (center marked 3, excluded)
_INDEX = np.array([0, 0, 0, 0, 0, 0, 0,
                   0, 1, 1, 1, 1, 1, 0,
                   0, 1, 2, 2, 2, 1, 0,
                   0, 1, 2, 3, 2, 1, 0,
                   0, 1, 2, 2, 2, 1, 0,
                   0, 1, 1, 1, 1, 1, 0,
                   0, 0, 0, 0, 0, 0, 0], dtype=np.int64).reshape(7, 7)

TAPS = []  # (t, ring, dy, dx) in DRAM plane order, center skipped
for i in range(K):
    for j in range(K):
        if i == 3 and j == 3:
            continue
        TAPS.append((len(TAPS), int(_INDEX[i, j]), 3 - i, 3 - j))

# maximal runs of taps (in t-order) sharing one ring -> one DVE mul each
RUNS = []  # [t_lo, t_hi, ring]
for t, r, dy, dx in TAPS:
    if RUNS and RUNS[-1][2] == r and RUNS[-1][1] == t:
        RUNS[-1][1] = t + 1
    else:
        RUNS.append([t, t + 1, r])
RUNS = [tuple(x) for x in RUNS]


def _band_matrix() -> np.ndarray:
    band = np.zeros((128, BANDW), dtype=np.float32)
    for p in range(128):
        band[p, p + C0] = 1.0
    return band


def _build():
    nc = bacc.Bacc("TRN2", target_bir_lowering=False, debug=False,
                   num_devices=N_CORES)
    # float32r: same 4-byte layout as fp32 (numpy sees float32). Static BIR
    # typing only — the in-place z multiply re-produces every element rounded
    # to fp32r before any matmul consumes it.
    aff = nc.dram_tensor("affinity", [B_CORE, NTAP, 128, 2, HWP], FP32R,
                         kind="ExternalInput").ap()
    att = nc.dram_tensor("attention", [B_CORE, 4, 128, 2, W], FP32,
                         kind="ExternalInput").ap()
    cs = nc.dram_tensor("current_segmentation", [B_CORE, 128, 2, W], FP32,
                        kind="ExternalInput").ap()
    co = nc.dram_tensor("coarse_segmentation", [B_CORE, 128, 2, W], FP32,
                        kind="ExternalInput").ap()
    band = nc.dram_tensor("band", [128, BANDW], FP32, kind="ExternalInput").ap()
    out = nc.dram_tensor("out", [B_CORE, 128, 2, W], FP32,
                         kind="ExternalOutput").ap()

    with tile.TileContext(nc) as tc:
        with tc.tile_pool(name="const", bufs=1) as cpool, \
             tc.tile_pool(name="z2", bufs=6) as zp2, \
             tc.tile_pool(name="z4", bufs=13) as zp4, \
             tc.tile_pool(name="azp", bufs=4) as azpool, \
             tc.tile_pool(name="inp", bufs=2) as ipool, \
             tc.tile_pool(name="ep", bufs=2) as epool, \
             tc.tile_pool(name="ps", bufs=2, space="PSUM") as pspool:

            bandf = cpool.tile([128, BANDW], FP32)
            nc.scalar.dma_start(out=bandf[:], in_=band[:, :])
            bandr = cpool.tile([128, BANDW], FP32R)
            nc.vector.tensor_copy(bandr[:], bandf[:])
            identb = cpool.tile([128, 128], BF16)
            nc.vector.tensor_copy(identb[:], bandf[:, C0:C0 + 128])
            identr = bandr[:, C0:C0 + 128]

            zpools = {2: zp2, 4: zp4}
            st = {}  # per-image state for the deferred epilogue

            def epilogue1(img):
                # needs psA+psU closed; overlaps the tail T matmuls
                s = st[img]
                e = epool.tile([128, 2, W], FP32, tag="e")
                s["e"] = e
                nc.scalar.activation(e[:], s["A"][:],
                                     mybir.ActivationFunctionType.Copy,
                                     bias=EPS)
                nc.vector.reciprocal_approx_fast(e[:], e[:])
                m2 = epool.tile([128, 2, W], FP32, tag="m2")
                s["m2"] = m2
                nc.vector.tensor_mul(m2[:], s["U"][:], s["cot"][:])
                nc.vector.tensor_mul(m2[:], m2[:], e[:])
                nc.vector.tensor_sub(m2[:], m2[:], s["cot"][:])

            def epilogue2(img):
                s = st[img]
                e, m2 = s["e"], s["m2"]
                nc.vector.tensor_mul(e[:], e[:], s["cst"][:])
                m1 = epool.tile([128, 2, W], FP32, tag="m1")
                nc.vector.tensor_mul(m1[:], s["T"][:], e[:])
                nc.vector.tensor_sub(m1[:], m1[:], m2[:])
                nc.scalar.dma_start(out=out[img], in_=m1[:])

            for img in range(B_CORE):
                attf = ipool.tile([128, 4, 2, W], FP32, tag="attf")
                nc.gpsimd.dma_start(out=attf[:, 0], in_=att[img, 0])
                nc.gpsimd.dma_start(out=attf[:, 3], in_=att[img, 3])
                cst = ipool.tile([128, 2, W], FP32, tag="cst")
                nc.gpsimd.dma_start(out=cst[:], in_=cs[img])
                cot = ipool.tile([128, 2, W], FP32, tag="cot")
                nc.gpsimd.dma_start(out=cot[:], in_=co[img])
                att3r = ipool.tile([128, 2, W], FP32R, tag="att3r")
                nc.scalar.activation(att3r[:], attf[:, 3],
                                     mybir.ActivationFunctionType.Copy)

                # affinity chunk DMAs: graded sizes so compute starts early;
                # all on the sync HWDGE ring, nothing else ever queues there
                zts = []
                for ci, (lo, hi) in enumerate(CHUNKS):
                    cn = hi - lo
                    zt = zpools[cn].tile([128, cn, 2, HWP], FP32R, tag="zt")
                    zts.append(zt)
                    nc.sync.dma_start(
                        out=zt[:],
                        in_=aff[img, lo:hi].transpose([1, 0, 2, 3]))
                    if ci == 0:
                        nc.gpsimd.dma_start(out=attf[:, 1], in_=att[img, 1])
                        nc.gpsimd.dma_start(out=attf[:, 2], in_=att[img, 2])

                # PSUM accumulators, opened with the +att3 tap
                psU = pspool.tile([128, 2, W], FP32, tag="U")
                psA = pspool.tile([128, 2, W], FP32, tag="A")
                psT = pspool.tile([128, 2, W], FP32, tag="T")
                st[img] = {"U": psU, "A": psA, "T": psT, "cst": cst,
                           "cot": cot}
                nc.tensor.matmul(out=psU[:], lhsT=identr, rhs=att3r[:],
                                 start=True, stop=False)
                nc.tensor.matmul(out=psA[:], lhsT=identr, rhs=att3r[:],
                                 start=True, stop=False)
                nc.tensor.matmul(out=psT[:], lhsT=identr, rhs=att3r[:],
                                 start=True, stop=False)

                for ci, (lo, hi) in enumerate(CHUNKS):
                    zt = zts[ci]
                    # z = att_r * aff (DVE, in-place, fp32r out)
                    for rlo, rhi, r in RUNS:
                        a, b = max(rlo, lo), min(rhi, hi)
                        if a >= b:
                            continue
                        zs = zt[:, a - lo:b - lo, :, PAD:PAD + W]
                        nc.vector.tensor_tensor(
                            out=zs,
                            in0=zs.bitcast(FP32),
                            in1=attf[:, r].unsqueeze(1).broadcast_to(
                                [128, b - a, 2, W]),
                            op=mybir.AluOpType.mult)
                    # |z| -> bf16 (ACT)
                    azt = azpool.tile([128, 4, 2, W], BF16, tag="az")
                    nc.scalar.activation(azt[:, 0:hi - lo],
                                         zt[:, :, :, PAD:PAD + W],
                                         mybir.ActivationFunctionType.Abs)
                    final = hi == NTAP

                    def mm_u():
                        for t, r, dy, dx in TAPS[lo:hi]:
                            nc.tensor.matmul(out=psU[:], lhsT=identr,
                                             rhs=zt[:, t - lo, :, PAD:PAD + W],
                                             start=False, stop=(t == NTAP - 1))

                    def mm_t():
                        for t, r, dy, dx in TAPS[lo:hi]:
                            if dy % 2 == 0:
                                s = dy // 2
                                nc.tensor.matmul(
                                    out=psT[:],
                                    lhsT=bandr[:, C0 + s:C0 + s + 128],
                                    rhs=zt[:, t - lo, :, PAD + dx:PAD + dx + W],
                                    start=False, stop=False)
                            else:
                                for h in (0, 1):
                                    s = ((dy - 1) // 2 if h == 0
                                         else (dy + 1) // 2)
                                    nc.tensor.matmul(
                                        out=psT[:, h],
                                        lhsT=bandr[:, C0 + s:C0 + s + 128],
                                        rhs=zt[:, t - lo, 1 - h,
                                               PAD + dx:PAD + dx + W],
                                        start=False,
                                        stop=(t == NTAP - 1 and h == 1))

                    def mm_a():
                        for t, r, dy, dx in TAPS[lo:hi]:
                            nc.tensor.matmul(out=psA[:], lhsT=identb,
                                             rhs=azt[:, t - lo],
                                             start=False, stop=(t == NTAP - 1))

                    if final and img == B_CORE - 1:
                        mm_u()
                        mm_a()
                        epilogue1(img)
                        mm_t()
                        epilogue2(img)
                    else:
                        mm_u()
                        mm_t()
                        mm_a()
                    if ci == EPI_AT and img > 0:
                        epilogue1(img - 1)
                        epilogue2(img - 1)

    nc.compile()
    return nc


_NC_CACHE = None


def _get_nc():
    global _NC_CACHE
    if _NC_CACHE is None:
        _NC_CACHE = _build()
    return _NC_CACHE


_KEEP = [k for k in range(49) if k != 24]


def run(inputs: dict, trace: bool = False):
    """Run on 8 NeuronCores; returns (out [16,1,256,256], BassKernelResults)."""
    aff = np.asarray(inputs["affinity"], dtype=np.float32)
    att = np.asarray(inputs["attention"], dtype=np.float32)
    cs = np.asarray(inputs["current_segmentation"], dtype=np.float32)
    co = np.asarray(inputs["coarse_segmentation"], dtype=np.float32)
    band = _band_matrix()

    nc = _get_nc()
    in_maps = []
    for c in range(N_CORES):
        s = slice(c * B_CORE, (c + 1) * B_CORE)
        affp = np.zeros((B_CORE, NTAP, 128, 2, HWP), np.float32)
        affp[..., PAD:PAD + W] = aff[s][:, _KEEP].reshape(
            B_CORE, NTAP, 128, 2, W)
        in_maps.append({
            "affinity": affp,
            "attention": np.ascontiguousarray(att[s]).reshape(
                B_CORE, 4, 128, 2, W),
            "current_segmentation": np.ascontiguousarray(cs[s]).reshape(
                B_CORE, 128, 2, W),
            "coarse_segmentation": np.ascontiguousarray(co[s]).reshape(
                B_CORE, 128, 2, W),
            "band": band,
        })
    last_err = None
    for attempt in range(3):
        try:
            res = run_bass_kernel_spmd(nc, in_maps, list(range(N_CORES)),
                                       trace=trace)
            break
        except Exception as e:  # transient NRT_EXEC_UNIT_UNRECOVERABLE flakes
            last_err = e
            import time
            time.sleep(10)
    else:
        raise last_err
    full = np.concatenate(
        [res.results[c]["out"].reshape(B_CORE, 1, H, W) for c in range(N_CORES)],
        axis=0)
    return full, res


def kernel(**inputs) -> np.ndarray:
    out, _ = run(inputs, trace=False)
    return out


# revision 7
# speedup vs baseline: 1.1382x; 1.0648x over previous
"""DYSPN attention-conv kernel for Trainium2 (8 NeuronCores, batch-parallel).

Math (the reference's unfold/fold pair collapses algebraically):
  per image, per tap k=(i,j) != center, ring r = INDEX[i,j], dy = 3-i, dx = 3-j:
    z_k[y,x]  = att_r[y,x] * aff_k[y,x]
    U[y,x]    = sum_k z_k[y,x]                       (S_ppt - att3)
    A[y,x]    = sum_k |z_k[y,x]|                     (S_prime - att3; att >= 0)
    T[y,x]    = sum_k z_k[y+dy, x+dx]  (in-image)    (fold7(z))
  out = r * ((T+att3)*cs - (U+att3)*co) + co,  r = 1/(A+att3+eps)

Layout: batch 16 -> 2 images/core. Row-pair layout: partition p holds image
rows {2p, 2p+1}; each affinity tap plane is host-padded to [128, 2, 262]
(3 zero cols each side per half-row) so column shifts read DRAM-zeroed
guards and every DMA descriptor is one contiguous 2096B run.
  - DVE: z = att*aff (ring-broadcast tensor_tensor, in-place, fp32r out)
  - ACT: |z| -> bf16 az tiles; PSUM A + eps copy
  - PE : U/A/T as banded-"identity" matmuls accumulating in PSUM
         (one full bank [128,2,256] per accumulator; even row shifts via
         band diagonal on pairs, odd shifts via two per-half matmuls)
  - DVE: epilogue, hoisted off the z-mult stream (img0's epilogue is issued
         mid-img1 so it never head-of-line-blocks img1's multiplies);
         out stores + cs/co loads ride the scalar HWDGE ring so the sync
         ring streams affinity uninterrupted.
"""
import sys

sys.path.insert(0, "/opt/trn_rl_repo")

import numpy as np

import concourse.bass as bass  # noqa: F401  (registers engines)
import concourse.tile as tile
from concourse import bacc, mybir
from concourse.bass_utils import run_bass_kernel_spmd

FP32 = mybir.dt.float32
FP32R = mybir.dt.float32r
BF16 = mybir.dt.bfloat16

N_CORES = 8
B_FULL = 16
B_CORE = B_FULL // N_CORES  # 2 images per core
H = W = 256
K = 7
NTAP = 48                 # 49 minus center
PAD = 3                   # zero guard cols per side of each half-row
HWP = 2 * PAD + W         # 262: padded half-row width
BANDW = 132               # band[p, q] = 1 iff q == p + C0
C0 = 2
EPS = 1e-6

CHUNK_SIZES = [2, 2] + [4] * 10 + [2, 2]
CHUNKS = []
_lo = 0
for _cs in CHUNK_SIZES:
    CHUNKS.append((_lo, _lo + _cs))
    _lo += _cs
EPI_AT = 3  # issue previous image's epilogue after this chunk index of the next

# ring index of each tap in the 7x7 window (center marked 3, excluded)
_INDEX = np.array([0, 0, 0, 0, 0, 0, 0,
                   0, 1, 1, 1, 1, 1, 0,
                   0, 1, 2, 2, 2, 1, 0,
                   0, 1, 2, 3, 2, 1, 0,
                   0, 1, 2, 2, 2, 1, 0,
                   0, 1, 1, 1, 1, 1, 0,
                   0, 0, 0, 0, 0, 0, 0], dtype=np.int64).reshape(7, 7)

TAPS = []  # (t, ring, dy, dx) in DRAM plane order, center skipped
for i in range(K):
    for j in range(K):
        if i == 3 and j == 3:
            continue
        TAPS.append((len(TAPS), int(_INDEX[i, j]), 3 - i, 3 - j))

# maximal runs of taps (in t-order) sharing one ring -> one DVE mul each
RUNS = []  # [t_lo, t_hi, ring]
for t, r, dy, dx in TAPS:
    if RUNS and RUNS[-1][2] == r and RUNS[-1][1] == t:
        RUNS[-1][1] = t + 1
    else:
        RUNS.append([t, t + 1, r])
RUNS = [tuple(x) for x in RUNS]


def _band_matrix() -> np.ndarray:
    band = np.zeros((128, BANDW), dtype=np.float32)
    for p in range(128):
        band[p, p + C0] = 1.0
    return band


def _build():
    nc = bacc.Bacc("TRN2", target_bir_lowering=False, debug=False,
                   num_devices=N_CORES)
    # float32r: same 4-byte layout as fp32 (numpy sees float32). Static BIR
    # typing only — the in-place z multiply re-produces every element rounded
    # to fp32r before any matmul consumes it.
    aff2 = nc.dram_tensor("aff2", [B_CORE, 4, 128, 2, 2, HWP], FP32R,
                          kind="ExternalInput").ap()
    aff4 = nc.dram_tensor("aff4", [B_CORE, 10, 128, 4, 2, HWP], FP32R,
                          kind="ExternalInput").ap()
    att = nc.dram_tensor("attention", [B_CORE, 128, 4, 2, W], FP32,
                         kind="ExternalInput").ap()
    cs = nc.dram_tensor("current_segmentation", [B_CORE, 128, 2, W], FP32,
                        kind="ExternalInput").ap()
    co = nc.dram_tensor("coarse_segmentation", [B_CORE, 128, 2, W], FP32,
                        kind="ExternalInput").ap()
    band = nc.dram_tensor("band", [128, BANDW], FP32, kind="ExternalInput").ap()
    out = nc.dram_tensor("out", [B_CORE, 128, 2, W], FP32,
                         kind="ExternalOutput").ap()

    with tile.TileContext(nc) as tc:
        with tc.tile_pool(name="const", bufs=1) as cpool, \
             tc.tile_pool(name="z2", bufs=6) as zp2, \
             tc.tile_pool(name="z4", bufs=13) as zp4, \
             tc.tile_pool(name="azp", bufs=4) as azpool, \
             tc.tile_pool(name="inp", bufs=2) as ipool, \
             tc.tile_pool(name="ep", bufs=2) as epool, \
             tc.tile_pool(name="ps", bufs=2, space="PSUM") as pspool:

            bandf = cpool.tile([128, BANDW], FP32)
            nc.scalar.dma_start(out=bandf[:], in_=band[:, :])
            bandr = cpool.tile([128, BANDW], FP32R)
            nc.vector.tensor_copy(bandr[:], bandf[:])
            identb = cpool.tile([128, 128], BF16)
            nc.vector.tensor_copy(identb[:], bandf[:, C0:C0 + 128])
            identr = bandr[:, C0:C0 + 128]

            zpools = {2: zp2, 4: zp4}
            st = {}  # per-image state for the deferred epilogue

            def epilogue1(img):
                # needs psA+psU closed; overlaps the tail T matmuls
                s = st[img]
                e = epool.tile([128, 2, W], FP32, tag="e")
                s["e"] = e
                nc.scalar.activation(e[:], s["A"][:],
                                     mybir.ActivationFunctionType.Copy,
                                     bias=EPS)
                nc.vector.reciprocal_approx_fast(e[:], e[:])
                m2 = epool.tile([128, 2, W], FP32, tag="m2")
                s["m2"] = m2
                nc.vector.tensor_mul(m2[:], s["U"][:], s["cot"][:])
                nc.vector.tensor_mul(m2[:], m2[:], e[:])
                nc.vector.tensor_sub(m2[:], m2[:], s["cot"][:])

            def epilogue2(img):
                s = st[img]
                e, m2 = s["e"], s["m2"]
                nc.vector.tensor_mul(e[:], e[:], s["cst"][:])
                m1 = epool.tile([128, 2, W], FP32, tag="m1")
                nc.vector.tensor_mul(m1[:], s["T"][:], e[:])
                nc.vector.tensor_sub(m1[:], m1[:], m2[:])
                nc.scalar.dma_start(out=out[img], in_=m1[:])

            for img in range(B_CORE):
                attf = ipool.tile([128, 4, 2, W], FP32, tag="attf")
                nc.gpsimd.dma_start(out=attf[:], in_=att[img])
                cst = ipool.tile([128, 2, W], FP32, tag="cst")
                nc.gpsimd.dma_start(out=cst[:], in_=cs[img])
                cot = ipool.tile([128, 2, W], FP32, tag="cot")
                nc.gpsimd.dma_start(out=cot[:], in_=co[img])
                att3r = ipool.tile([128, 2, W], FP32R, tag="att3r")
                nc.scalar.activation(att3r[:], attf[:, 3],
                                     mybir.ActivationFunctionType.Copy)

                # affinity chunk DMAs: graded sizes so compute starts early;
                # all on the sync HWDGE ring, nothing else ever queues there
                zts = []
                i2 = i4 = 0
                for ci, (lo, hi) in enumerate(CHUNKS):
                    cn = hi - lo
                    zt = zpools[cn].tile([128, cn, 2, HWP], FP32R, tag="zt")
                    zts.append(zt)
                    if cn == 2:
                        src_ap = aff2[img, i2]
                        i2 += 1
                    else:
                        src_ap = aff4[img, i4]
                        i4 += 1
                    nc.sync.dma_start(out=zt[:], in_=src_ap)

                # PSUM accumulators, opened with the +att3 tap
                psU = pspool.tile([128, 2, W], FP32, tag="U")
                psA = pspool.tile([128, 2, W], FP32, tag="A")
                psT = pspool.tile([128, 2, W], FP32, tag="T")
                st[img] = {"U": psU, "A": psA, "T": psT, "cst": cst,
                           "cot": cot}
                nc.tensor.matmul(out=psU[:], lhsT=identr, rhs=att3r[:],
                                 start=True, stop=False)
                nc.tensor.matmul(out=psA[:], lhsT=identr, rhs=att3r[:],
                                 start=True, stop=False)
                nc.tensor.matmul(out=psT[:], lhsT=identr, rhs=att3r[:],
                                 start=True, stop=False)

                for ci, (lo, hi) in enumerate(CHUNKS):
                    zt = zts[ci]
                    # z = att_r * aff (DVE, in-place, fp32r out)
                    for rlo, rhi, r in RUNS:
                        a, b = max(rlo, lo), min(rhi, hi)
                        if a >= b:
                            continue
                        zs = zt[:, a - lo:b - lo, :, PAD:PAD + W]
                        nc.vector.tensor_tensor(
                            out=zs,
                            in0=zs.bitcast(FP32),
                            in1=attf[:, r].unsqueeze(1).broadcast_to(
                                [128, b - a, 2, W]),
                            op=mybir.AluOpType.mult)
                    # |z| -> bf16 (ACT)
                    azt = azpool.tile([128, 4, 2, W], BF16, tag="az")
                    nc.scalar.activation(azt[:, 0:hi - lo],
                                         zt[:, :, :, PAD:PAD + W],
                                         mybir.ActivationFunctionType.Abs)
                    final = hi == NTAP

                    def mm_u():
                        for t, r, dy, dx in TAPS[lo:hi]:
                            nc.tensor.matmul(out=psU[:], lhsT=identr,
                                             rhs=zt[:, t - lo, :, PAD:PAD + W],
                                             start=False, stop=(t == NTAP - 1))

                    def mm_t():
                        for t, r, dy, dx in TAPS[lo:hi]:
                            if dy % 2 == 0:
                                s = dy // 2
                                nc.tensor.matmul(
                                    out=psT[:],
                                    lhsT=bandr[:, C0 + s:C0 + s + 128],
                                    rhs=zt[:, t - lo, :, PAD + dx:PAD + dx + W],
                                    start=False, stop=False)
                            else:
                                for h in (0, 1):
                                    s = ((dy - 1) // 2 if h == 0
                                         else (dy + 1) // 2)
                                    nc.tensor.matmul(
                                        out=psT[:, h],
                                        lhsT=bandr[:, C0 + s:C0 + s + 128],
                                        rhs=zt[:, t - lo, 1 - h,
                                               PAD + dx:PAD + dx + W],
                                        start=False,
                                        stop=(t == NTAP - 1 and h == 1))

                    def mm_a():
                        for t, r, dy, dx in TAPS[lo:hi]:
                            nc.tensor.matmul(out=psA[:], lhsT=identb,
                                             rhs=azt[:, t - lo],
                                             start=False, stop=(t == NTAP - 1))

                    if final and img == B_CORE - 1:
                        mm_u()
                        mm_a()
                        epilogue1(img)
                        mm_t()
                        epilogue2(img)
                    else:
                        mm_u()
                        mm_t()
                        mm_a()
                    if ci == EPI_AT and img > 0:
                        epilogue1(img - 1)
                        epilogue2(img - 1)

    nc.compile()
    return nc


_NC_CACHE = None


def _get_nc():
    global _NC_CACHE
    if _NC_CACHE is None:
        _NC_CACHE = _build()
    return _NC_CACHE


_KEEP = [k for k in range(49) if k != 24]


def run(inputs: dict, trace: bool = False):
    """Run on 8 NeuronCores; returns (out [16,1,256,256], BassKernelResults)."""
    aff = np.asarray(inputs["affinity"], dtype=np.float32)
    att = np.asarray(inputs["attention"], dtype=np.float32)
    cs = np.asarray(inputs["current_segmentation"], dtype=np.float32)
    co = np.asarray(inputs["coarse_segmentation"], dtype=np.float32)
    band = _band_matrix()

    nc = _get_nc()
    t2 = [0, 1, 2, 3, 44, 45, 46, 47]   # taps of the four 2-tap chunks
    in_maps = []
    for c in range(N_CORES):
        s = slice(c * B_CORE, (c + 1) * B_CORE)
        aff_k = aff[s][:, _KEEP].reshape(B_CORE, NTAP, 128, 2, W)
        a2 = np.zeros((B_CORE, 4, 128, 2, 2, HWP), np.float32)
        a2[..., PAD:PAD + W] = aff_k[:, t2].reshape(
            B_CORE, 4, 2, 128, 2, W).transpose(0, 1, 3, 2, 4, 5)
        a4 = np.zeros((B_CORE, 10, 128, 4, 2, HWP), np.float32)
        a4[..., PAD:PAD + W] = aff_k[:, 4:44].reshape(
            B_CORE, 10, 4, 128, 2, W).transpose(0, 1, 3, 2, 4, 5)
        in_maps.append({
            "aff2": a2,
            "aff4": a4,
            "attention": np.ascontiguousarray(att[s]).reshape(
                B_CORE, 4, 128, 2, W).transpose(0, 2, 1, 3, 4).copy(),
            "current_segmentation": np.ascontiguousarray(cs[s]).reshape(
                B_CORE, 128, 2, W),
            "coarse_segmentation": np.ascontiguousarray(co[s]).reshape(
                B_CORE, 128, 2, W),
            "band": band,
        })
    last_err = None
    for attempt in range(3):
        try:
            res = run_bass_kernel_spmd(nc, in_maps, list(range(N_CORES)),
                                       trace=trace)
            break
        except Exception as e:  # transient NRT_EXEC_UNIT_UNRECOVERABLE flakes
            last_err = e
            import time
            time.sleep(10)
    else:
        raise last_err
    full = np.concatenate(
        [res.results[c]["out"].reshape(B_CORE, 1, H, W) for c in range(N_CORES)],
        axis=0)
    return full, res


def kernel(**inputs) -> np.ndarray:
    out, _ = run(inputs, trace=False)
    return out
